# revision 6
# baseline (speedup 1.0000x reference)
"""DWT (db4-style, depthwise stride-2, reflect-pad) layer as a Trainium2
Bass/Tile kernel.

Math: for input x[B, T, C] and 8-tap filters lo/hi the reference computes a
reflect-pad-7, stride-2, depthwise cross-correlation cropped by 3 per side:

    out[b, t', c]     = sum_k lo[k] * xe[b, 2 t' + k, c]
    out[b, t', C + c] = sum_k hi[k] * xe[b, 2 t' + k, c]

with xe[u] = x[u - 1] for u in [1, T+1), xe[0] = x[1], xe[T+1] = x[T-2]
(after the crop only one reflected element is needed per side), and
t' in [0, T/2 - 2).

Device mapping (per core), redesigned from the banded-polyphase baseline:
  - host pre-transposes x to [T, Bl, C] so each time step is a 1 KB
    contiguous row; time goes on the SBUF partition axis one step per
    partition (no polyphase).
  - ONE stationary matrix W[128, 122] holds BOTH filters as stride-2
    bands: W[2m+k, m] = lo[k], W[2m+k, 61+m] = hi[k].  A single f32r
    matmul per block-pair (rhs [128, 2*256]) produces 61 lo rows and 61
    hi rows for two 61-output blocks at once -- 2x fewer PE rows than
    separate lo/hi matmuls, and f32r runs 4x faster than fp32 at
    free-dim >= 256.
  - loads are batched: one HWDGE DMA with a custom overlapping access
    pattern [[256,128],[122*256,H],[1,256]] brings in H=8 block windows
    (1 MB) per dispatch -- 17 load dispatches/core instead of 66.
  - PSUM pairs [122, 512] are evacuated whole (no lo/hi interleave) by
    alternating DVE/Act copies that downcast to bf16 into a staging tile;
    lo and hi go to SEPARATE bf16 DRAM outputs (partition ranges 0:61 /
    61:122 of the same staging tile), 4 pairs per SWDGE store DMA.
    bf16 halves store HBM traffic (tolerance is 2e-2, bf16 adds ~2e-3).
  - the host reassembles: upcast + concat([lo, hi], axis=-1) + transpose.

Sharding: data-parallel over batch, 4 batches per core on 8 cores.
"""

import numpy as np

import concourse.bacc as bacc
import concourse.mybir as mybir
import concourse.tile as tile
from concourse.ap import AP
from concourse.bass_utils import run_bass_kernel_spmd

F32 = mybir.dt.float32
F32R = mybir.dt.float32r
BF16 = mybir.dt.bfloat16

B, T, C = 32, 16384, 64
N_CORES = 8
BL = B // N_CORES   # 4 batches per core
NF = BL * C         # 256 floats = 1 KB per time step
M = 61              # outputs per block (2M+6 = 128-step window)
NOUT = T // 2 - 2   # 8190
NBLK = 134          # full blocks; NBLK*M = 8174
TAIL_T0 = NBLK * M  # 8174
TAIL_N = NOUT - TAIL_T0  # 16
H = 8               # blocks per load supertile / store group (S = H//2 pairs)


def _build_nc(store_dtype=BF16, mm_dtype=F32R):
    nc = bacc.Bacc("TRN2", target_bir_lowering=False, debug=False)
    x_d = nc.dram_tensor("x", [T, BL, C], F32, kind="ExternalInput")
    w_d = nc.dram_tensor("w", [128, 122], F32, kind="ExternalInput")
    lo_d = nc.dram_tensor("lo", [NOUT, NF], store_dtype, kind="ExternalOutput")
    hi_d = nc.dram_tensor("hi", [NOUT, NF], store_dtype, kind="ExternalOutput")

    n_super = (NBLK + H - 1) // H  # 17: 16 full + 1 with 6 blocks

    with tile.TileContext(nc) as tc:
        with (
            tc.tile_pool(name="wpool", bufs=1) as wpool,
            tc.tile_pool(name="xin", bufs=3) as xpool,
            tc.tile_pool(name="oout", bufs=3) as opool,
            tc.tile_pool(name="ps", bufs=6, space="PSUM") as pspool,
        ):
            # tiles feeding the PE are declared mm_dtype (f32r) and the DMAs
            # bitcast their sources to match: walrus requires f32r matmult
            # inputs to be *produced* as f32r, not just viewed that way.
            w_t = wpool.tile([128, 122], mm_dtype)
            nc.sync.dma_start(out=w_t[:], in_=w_d[:].bitcast(mm_dtype))
            w_r = w_t[:]

            pair_ctr = 0
            for s in range(n_super):
                q0 = H * s
                hs = min(H, NBLK - q0)  # 8, last is 6
                # xt[p, h, :] = xe[122*(q0+h) + p]; xe[u] = x[u-1] interior
                xt = xpool.tile([128, H * NF], mm_dtype, tag="xt")
                xv = xt[:].rearrange("p (h w) -> p h w", h=H)
                if s == 0:
                    # partitions 1..127 interior for all h
                    nc.sync.dma_start(
                        out=xv[1:128, 0:hs],
                        in_=AP(x_d[:].tensor, 0,
                               [[NF, 127], [122 * NF, hs], [1, NF]])
                        .bitcast(mm_dtype))
                    # partition 0, h>=1: xe[122h] = x[122h - 1]
                    nc.sync.dma_start(
                        out=xv[0:1, 1:hs],
                        in_=AP(x_d[:].tensor, (122 - 1) * NF,
                               [[0, 1], [122 * NF, hs - 1], [1, NF]])
                        .bitcast(mm_dtype))
                    # partition 0, h=0: xe[0] = x[1]
                    nc.sync.dma_start(
                        out=xv[0:1, 0:1],
                        in_=x_d[1:2].rearrange("t b c -> t (b c)").unsqueeze(1)
                        .bitcast(mm_dtype))
                else:
                    nc.sync.dma_start(
                        out=xv[:, 0:hs],
                        in_=AP(x_d[:].tensor, (122 * q0 - 1) * NF,
                               [[NF, 128], [122 * NF, hs], [1, NF]])
                        .bitcast(mm_dtype))

                npair = hs // 2  # 4, last 3
                ot = opool.tile([122, (H // 2) * 2 * NF], store_dtype, tag="ot")
                for p in range(npair):
                    ps = pspool.tile([122, 2 * NF], F32, tag="ps")
                    rhs = xt[:, 2 * p * NF:(2 * p + 2) * NF]
                    nc.tensor.matmul(out=ps[:], lhsT=w_r, rhs=rhs)
                    dst = ot[:, p * 2 * NF:(p + 1) * 2 * NF]
                    if pair_ctr % 2 == 0:
                        nc.vector.tensor_copy(out=dst, in_=ps[:])
                    else:
                        nc.scalar.copy(out=dst, in_=ps[:])
                    pair_ctr += 1

                # stores: rows t' = 61*q0 + 122*p + 61*h + m
                tg0 = M * q0
                ov = ot[:].rearrange("p (s h w) -> p s h w", h=2, w=NF)
                for f, f_d in ((0, lo_d), (1, hi_d)):
                    nc.gpsimd.dma_start(
                        out=f_d[tg0:tg0 + 2 * M * npair]
                        .rearrange("(s h m) w -> m s h w", h=2, m=M),
                        in_=ov[61 * f:61 * f + 61, 0:npair])

            # tail: outputs 8174..8189 (16), window xe[16348 .. 16386)
            u0 = 2 * TAIL_T0
            xt_t = xpool.tile([38, NF], mm_dtype, tag="xtt", bufs=1)
            nc.sync.dma_start(
                out=xt_t[0:37],
                in_=x_d[u0 - 1:u0 - 1 + 37].rearrange("t b c -> t (b c)")
                .bitcast(mm_dtype))
            nc.sync.dma_start(
                out=xt_t[37:38],
                in_=x_d[T - 2:T - 1].rearrange("t b c -> t (b c)")
                .bitcast(mm_dtype))
            ps_t = pspool.tile([122, NF], F32, tag="pst", bufs=1)
            nc.tensor.matmul(out=ps_t[:], lhsT=w_t[0:38, :], rhs=xt_t[:])
            ot_t = opool.tile([122, NF], store_dtype, tag="ott", bufs=1)
            nc.vector.tensor_copy(out=ot_t[:], in_=ps_t[:])
            nc.gpsimd.dma_start(out=lo_d[TAIL_T0:NOUT], in_=ot_t[0:TAIL_N])
            nc.gpsimd.dma_start(out=hi_d[TAIL_T0:NOUT],
                                in_=ot_t[61:61 + TAIL_N])

    nc.compile()
    return nc


def _build_w(dec_lo: np.ndarray, dec_hi: np.ndarray) -> np.ndarray:
    """Combined banded stationary matrix [128, 122]: cols 0:61 lo, 61:122 hi."""
    lo = np.asarray(dec_lo, np.float32)
    hi = np.asarray(dec_hi, np.float32)
    w = np.zeros((128, 122), np.float32)
    for m in range(M):
        for k in range(8):
            w[2 * m + k, m] = lo[k]
            w[2 * m + k, 61 + m] = hi[k]
    return w


_NC_CACHE = {}


def _get_nc():
    key = "v2"
    if key not in _NC_CACHE:
        _NC_CACHE[key] = _build_nc()
    return _NC_CACHE[key]


def kernel(x: np.ndarray, dec_lo: np.ndarray, dec_hi: np.ndarray) -> np.ndarray:
    x = np.asarray(x, np.float32)
    assert x.shape == (B, T, C), x.shape
    nc = _get_nc()
    w = _build_w(dec_lo, dec_hi)
    in_maps = [
        {"x": np.ascontiguousarray(x[i * BL:(i + 1) * BL].transpose(1, 0, 2)),
         "w": w}
        for i in range(N_CORES)
    ]
    res = run_bass_kernel_spmd(nc, in_maps, core_ids=list(range(N_CORES)))
    out = np.empty((B, NOUT, 2 * C), np.float32)
    for i in range(N_CORES):
        lo = np.asarray(res.results[i]["lo"]).astype(np.float32)
        hi = np.asarray(res.results[i]["hi"]).astype(np.float32)
        full = np.concatenate(
            [lo.reshape(NOUT, BL, C), hi.reshape(NOUT, BL, C)], axis=-1)
        out[i * BL:(i + 1) * BL] = full.transpose(1, 0, 2)
    return out


# revision 7
# speedup vs baseline: 3.8686x; 3.8686x over previous
"""DWT (db4-style, depthwise stride-2, reflect-pad) layer as a Trainium2
Bass/Tile kernel.

Math: for input x[B, T, C] and 8-tap filters lo/hi the reference computes a
reflect-pad-7, stride-2, depthwise cross-correlation cropped by 3 per side:

    out[b, t', c]     = sum_k lo[k] * xe[b, 2 t' + k, c]
    out[b, t', C + c] = sum_k hi[k] * xe[b, 2 t' + k, c]

with xe[u] = x[u - 1] for u in [1, T+1), xe[0] = x[1], xe[T+1] = x[T-2]
(after the crop only one reflected element is needed per side), and
t' in [0, T/2 - 2).

Device mapping (per core).  The binding resource on this part is DMA
DESCRIPTORS: the HWDGE generates ~50M descriptors/s (shared), SWDGE ~8.5
ns/desc serial on GpSimd, and descriptors below 2 KB also waste DMA-engine
cycles.  So both sides of the kernel are laid out for few, fat descriptors:

  - time on the SBUF partition axis one step per partition; ONE stationary
    matrix W[128, 122] holds BOTH filters as stride-2 bands (W[2m+k, m] =
    lo[k], W[2m+k, 61+m] = hi[k]).  One f32r matmul per pair of 61-output
    blocks (rhs [128, 512]) -- f32r is 4x faster than fp32 at free >= 256.
  - loads: the HOST pre-tiles xe = [x[1], x, x[T-2]] into supertile-major
    x_t[s, p, h*256] = xe[122*(16s+h) + p], so each SBUF partition line of
    a supertile is one CONTIGUOUS 16 KB run in DRAM: 128 descriptors per
    1.9 MB load, ~1.2K load descriptors per core total.
  - stores: outputs go to DRAM in q-major order lo_dev[m, q, (b,c)] with
    t' = 61q + m, so each store descriptor covers a supertile's worth of
    q per partition (8 KB): 61 descriptors per store, SWDGE (gpsimd).
    Outputs are bf16 (tolerance is 2e-2, bf16 adds ~2e-3) halving store
    bytes; lo/hi are separate tensors so PSUM [122, 512] is evacuated by
    ONE whole-tile DVE/Act copy (no interleave, no partition shifts).
  - the host un-permutes (transpose+reshape), upcasts, and concatenates.

Sharding: data-parallel over batch, 4 batches per core on 8 cores.
"""

import numpy as np

import concourse.bacc as bacc
import concourse.mybir as mybir
import concourse.tile as tile
from concourse.bass_utils import run_bass_kernel_spmd

F32 = mybir.dt.float32
F32R = mybir.dt.float32r
BF16 = mybir.dt.bfloat16

B, T, C = 32, 16384, 64
N_CORES = 8
BL = B // N_CORES   # 4 batches per core
NF = BL * C         # 256 floats = 1 KB per time step
M = 61              # outputs per block (2M+6 = 128-step window)
NOUT = T // 2 - 2   # 8190
NBLK = 134          # full blocks; NBLK*M = 8174
TAIL_T0 = NBLK * M  # 8174
TAIL_N = NOUT - TAIL_T0  # 16
H = 16              # blocks per load supertile / store group
NSUP = (NBLK + H - 1) // H  # 9: 8 full + 1 with 6 blocks
NQ = NBLK + 1       # q dim of the output tensors (tail block is q=134)


def _build_nc(store_dtype=BF16, mm_dtype=F32R):
    nc = bacc.Bacc("TRN2", target_bir_lowering=False, debug=False)
    x_d = nc.dram_tensor("x", [NSUP, 128, H * NF], F32, kind="ExternalInput")
    xt_d = nc.dram_tensor("xtail", [38, NF], F32, kind="ExternalInput")
    w_d = nc.dram_tensor("w", [128, 122], F32, kind="ExternalInput")
    lo_d = nc.dram_tensor("lo", [M, NQ, NF], store_dtype, kind="ExternalOutput")
    hi_d = nc.dram_tensor("hi", [M, NQ, NF], store_dtype, kind="ExternalOutput")

    with tile.TileContext(nc) as tc:
        with (
            tc.tile_pool(name="wpool", bufs=1) as wpool,
            tc.tile_pool(name="xin", bufs=3) as xpool,
            tc.tile_pool(name="oout", bufs=3) as opool,
            tc.tile_pool(name="ps", bufs=7, space="PSUM") as pspool,
        ):
            # tiles feeding the PE are declared mm_dtype (f32r); DMAs bitcast
            # their sources to match (walrus requires f32r matmult inputs to
            # be produced as f32r, not just viewed that way).
            w_t = wpool.tile([128, 122], mm_dtype)
            nc.sync.dma_start(out=w_t[:], in_=w_d[:].bitcast(mm_dtype))

            pair_ctr = 0
            for s in range(NSUP):
                q0 = H * s
                hs = min(H, NBLK - q0)  # 16, last is 6
                xt = xpool.tile([128, H * NF], mm_dtype, tag="xt")
                nc.sync.dma_start(out=xt[:, 0:hs * NF],
                                  in_=x_d[s, :, 0:hs * NF].bitcast(mm_dtype))

                st = opool.tile([122, H * NF], store_dtype, tag="st")
                for p in range(hs // 2):
                    ps = pspool.tile([122, 2 * NF], F32, tag="ps")
                    rhs = xt[:, 2 * p * NF:(2 * p + 2) * NF]
                    nc.tensor.matmul(out=ps[:], lhsT=w_t[:], rhs=rhs)
                    dst = st[:, 2 * p * NF:(2 * p + 2) * NF]
                    if pair_ctr % 2 == 0:
                        nc.vector.tensor_copy(out=dst, in_=ps[:])
                    else:
                        nc.scalar.copy(out=dst, in_=ps[:])
                    pair_ctr += 1

                # one fat-descriptor store per filter: partition m's data for
                # q in [q0, q0+hs) is one contiguous hs*512B run in DRAM
                sv = st[:].rearrange("p (q w) -> p q w", w=NF)
                nc.gpsimd.dma_start(out=lo_d[:, q0:q0 + hs, :],
                                    in_=sv[0:M, 0:hs])
                nc.gpsimd.dma_start(out=hi_d[:, q0:q0 + hs, :],
                                    in_=sv[M:2 * M, 0:hs])

            # tail block: outputs 8174..8189 (16) stored at q=134
            xt_t = xpool.tile([38, NF], mm_dtype, tag="xtt", bufs=1)
            nc.sync.dma_start(out=xt_t[:], in_=xt_d[:].bitcast(mm_dtype))
            ps_t = pspool.tile([122, NF], F32, tag="pst", bufs=1)
            nc.tensor.matmul(out=ps_t[:], lhsT=w_t[0:38, :], rhs=xt_t[:])
            st_t = opool.tile([122, NF], store_dtype, tag="stt", bufs=1)
            nc.vector.tensor_copy(out=st_t[:], in_=ps_t[:])
            nc.gpsimd.dma_start(out=lo_d[0:TAIL_N, NBLK, :],
                                in_=st_t[0:TAIL_N])
            nc.gpsimd.dma_start(out=hi_d[0:TAIL_N, NBLK, :],
                                in_=st_t[M:M + TAIL_N])

    nc.compile()
    return nc


def _build_w(dec_lo: np.ndarray, dec_hi: np.ndarray) -> np.ndarray:
    """Combined banded stationary matrix [128, 122]: cols 0:61 lo, 61:122 hi."""
    lo = np.asarray(dec_lo, np.float32)
    hi = np.asarray(dec_hi, np.float32)
    w = np.zeros((128, 122), np.float32)
    for m in range(M):
        for k in range(8):
            w[2 * m + k, m] = lo[k]
            w[2 * m + k, 61 + m] = hi[k]
    return w


def _prep_core(x: np.ndarray, i: int) -> dict:
    """Host-side input prep for core i: supertile-tiled xe + tail window."""
    xc = np.ascontiguousarray(
        x[i * BL:(i + 1) * BL].transpose(1, 0, 2)).reshape(T, NF)
    xe = np.concatenate([xc[1:2], xc, xc[T - 2:T - 1]], axis=0)  # [T+2, NF]
    # win[q, p, :] = xe[122q + p]
    win = np.lib.stride_tricks.as_strided(
        xe, shape=(NBLK, 128, NF),
        strides=(122 * xe.strides[0], xe.strides[0], xe.strides[1]))
    x_t = np.zeros((NSUP, 128, H * NF), np.float32)
    for s in range(NSUP):
        hs = min(H, NBLK - H * s)
        x_t[s, :, 0:hs * NF] = (
            win[H * s:H * s + hs].transpose(1, 0, 2).reshape(128, hs * NF))
    x_tail = np.ascontiguousarray(xe[2 * TAIL_T0:2 * TAIL_T0 + 38])
    return {"x": x_t, "xtail": x_tail}


_NC_CACHE = {}


def _get_nc():
    key = "v3"
    if key not in _NC_CACHE:
        _NC_CACHE[key] = _build_nc()
    return _NC_CACHE[key]


def kernel(x: np.ndarray, dec_lo: np.ndarray, dec_hi: np.ndarray) -> np.ndarray:
    x = np.asarray(x, np.float32)
    assert x.shape == (B, T, C), x.shape
    nc = _get_nc()
    w = _build_w(dec_lo, dec_hi)
    in_maps = []
    for i in range(N_CORES):
        m = _prep_core(x, i)
        m["w"] = w
        in_maps.append(m)
    res = run_bass_kernel_spmd(nc, in_maps, core_ids=list(range(N_CORES)))
    out = np.empty((B, NOUT, 2 * C), np.float32)
    for i in range(N_CORES):
        # [M, NQ, NF] q-major -> t' = 61q + m ordered [NOUT, BL, C]
        lo = np.asarray(res.results[i]["lo"]).astype(np.float32)
        hi = np.asarray(res.results[i]["hi"]).astype(np.float32)
        lo = lo.transpose(1, 0, 2).reshape(NQ * M, BL, C)[:NOUT]
        hi = hi.transpose(1, 0, 2).reshape(NQ * M, BL, C)[:NOUT]
        out[i * BL:(i + 1) * BL] = np.concatenate(
            [lo, hi], axis=-1).transpose(1, 0, 2)
    return out
